# revision 27
# baseline (speedup 1.0000x reference)
"""Trainium2 Bass kernel for nn_BGNLLLoss (bivariate-Gaussian NLL loss).

Math (per element t,p):
    mux,muy,lsx,lsy,pc = params[t,p,:];  x,y = targets[t,p,:]
    sx=e^lsx, sy=e^lsy, c=tanh(pc), nr=1-c^2
    a=(x-mux)/sx, b=(y-muy)/sy
    nll = min( (a^2+b^2-2abc)/(2nr) + lsx+lsy + 0.5 ln(nr) + ln(2pi),
               -ln(1e-20) )
    loss[p] = sum_t nll[t,p]

tanh-free reformulation (keeps ScalarE in ONE table set: exp+ln+square):
  t4  = e^{-2 pc}            =>  c = (1-t4)/(1+t4),  nr = 4 t4/(1+t4)^2
  gv  = a(1+t4) + b(t4-1)    =  (a - cb)(1+t4)
  (a^2+b^2-2abc)/(2nr)       =  gv^2 e^{2pc}/8 + b^2/2
  0.5 ln(nr)                 =  ln2 - pc - ln(1+t4)
  nll = min( (gv st)^2 + bh^2 + (lsx+lsy-pc) - lvc, K )
    with st = e^{pc}/(2 sqrt2), bh = b/sqrt2,
         lvc = ln(1+t4) - (ln2 + ln 2pi)  [folded into the Ln's scale/bias]

Engine split (per 256-row block; all 16 blocks pipelined by Tile):
  ScalarE: isx, isyh(=isy/sqrt2), t4, st, lvc          (5 ACTIVATEs)
  GpSimd : ny, s1=lsx+lsy, s1b=s1-pc                   (3 tensor ops)
  VectorE: bf16 2x chain a,bh,av,qn,gv,gvs,u,b2,W,V + 2 ts + 1 custom min
  TensorE: frame sum   acc[1,512] += ones^T @ nll      (2 matmuls)
Sharding: person dim split across 8 cores (512 each), no collectives.
"""

import json
import math
import os
import shutil
import tempfile
from contextlib import ExitStack

import numpy as np

import concourse.bass as bass
import concourse.bacc as bacc
import concourse.mybir as mybir
import concourse.tile as tile
from concourse import bass_utils
from concourse.dve_spec import Spec, Src0, Src1, C0, C1, lower, sq, minn, _has_src1
from concourse.dve_uop import DveOpSpec
import concourse.dve_ops as dve_ops

F32 = mybir.dt.float32
BF16 = mybir.dt.bfloat16
AF = mybir.ActivationFunctionType
ALU = mybir.AluOpType

T = 4096
P = 4096
N_CORES = 8
PC = P // N_CORES          # persons per core = 512
K = 2                      # 128-row subtiles per block
RB = 128 * K               # rows per block
NB = T // RB               # 16 blocks
TGT_W = PC * 2             # 1024
PRM_W = PC * 5             # 2560

LOG2PI = math.log(2.0 * math.pi)
LN2 = math.log(2.0)
CADD = LN2 + LOG2PI                    # additive const inside the min
CLAMP = -math.log(1e-20)               # 46.0517...
SQRT2 = math.sqrt(2.0)
B_ISYH = -0.5 * LN2                    # exp bias: isy/sqrt(2)
B_ST = -1.5 * LN2                      # exp bias: e^{pc}/(2 sqrt 2)
SC_LN = math.exp(-CADD)                # ln scale/bias: ln(1+t4) - CADD


# --------------------------------------------------------------------------
# Custom DVE op: out = min(in0 + in1 + s0, s1)
# --------------------------------------------------------------------------
def _register_dve_op(name: str, spec: Spec, subdim: bool = False):
    if name in dve_ops._SUB_OPCODE_FOR_NAME:
        return next(op for op in dve_ops.OPS if op.name == name)
    shas = {}
    for ver in ("v3", "v4"):
        uops = lower(spec, ver=ver)
        shas[ver] = DveOpSpec(
            name=name, opcode=0, uops=uops, rd1_en=_has_src1(spec)
        ).sha(ver)
    op = dve_ops.DveOp(name, spec, subdim=subdim, uops_sha=shas)
    dve_ops.OPS.append(op)
    dve_ops._SUB_OPCODE_FOR_NAME[name] = (
        dve_ops._CUSTOM_DVE_ROW_BASE + len(dve_ops.OPS) - 1
    )
    dve_ops.CUSTOM_DVE_SPECS[name] = spec
    return op


ADDMIN = _register_dve_op(
    "ADDMIN_BGNLL",
    Spec(
        body=minn(Src0 + Src1 + C0, C1),
        reference=lambda in0, in1, s0, s1, imm2: np.minimum(
            in0.astype(np.float32) + in1 + s0, s1
        ).astype(np.float32),
    ),
)

# out = sq(in0) + sq(in1)
SQ2 = _register_dve_op(
    "SQ2_BGNLL",
    Spec(
        body=sq(Src0) + sq(Src1),
        reference=lambda in0, in1, s0, s1, imm2: (
            np.square(in0.astype(np.float32)) + np.square(in1.astype(np.float32))
        ).astype(np.float32),
    ),
)

# Fast-log constants: for x = 2^e (1+f), int_bits(x)/2^23 = e + 127 + f and
# log2(x) = e + log2(1+f), so ln(x) ~= (int_bits(x) - SIGMA) * ln2/2^23 with
# the mantissa correction c = E[log2(1+f) - f] = 1.5 - 1/ln2 (zero-mean over
# uniform f) and the additive constant CADD both folded into SIGMA.
LNK = math.log(2.0) / (1 << 23)
_C_MEAN = 1.5 - 1.0 / math.log(2.0)            # 0.0573049...
SIGMA_F = (127.0 - _C_MEAN + CADD / math.log(2.0)) * (1 << 23)



# --------------------------------------------------------------------------
# ACT table-set fix: walrus assigns Exp -> exp_and_others and Ln ->
# natural_log_exp_and_others, reloading tables every block (~2.6us/block).
# Reorder act_info.json so the combined exp+ln set is found first for both.
# --------------------------------------------------------------------------
def _install_act_json():
    if os.environ.get("BGNLL_NO_ACT_JSON"):
        return
    if os.environ.get("BASS_ACT_ROOT_JSON_PATH"):
        return
    try:
        from neuronxcc.driver.Job import Job
        from neuronxcc.driver.jobs.support.FindActInfo import findActInfoFile
        src = findActInfoFile(Job.getPackageDir(), "gen3")
    except Exception:
        return
    if not src:
        return
    src_dir = os.path.dirname(src)
    dst_dir = os.path.join(tempfile.gettempdir(), "bgnll_act_root")
    os.makedirs(dst_dir, exist_ok=True)
    with open(src) as f:
        info = json.load(f)
    sets = info.get("act_func_sets", [])
    pref = [s for s in sets if s.get("name") == "natural_log_exp_and_others"]
    rest = [s for s in sets if s.get("name") != "natural_log_exp_and_others"]
    if not pref:
        return
    info["act_func_sets"] = pref + rest
    for name in os.listdir(src_dir):
        s = os.path.join(src_dir, name)
        d = os.path.join(dst_dir, name)
        if os.path.isfile(s) and not os.path.exists(d) and name != "act_info.json":
            try:
                os.symlink(s, d)
            except OSError:
                shutil.copy(s, d)
    with open(os.path.join(dst_dir, "act_info.json"), "w") as f:
        json.dump(info, f)
    os.environ["BASS_ACT_ROOT_JSON_PATH"] = os.path.join(dst_dir, "act_info.json")


# --------------------------------------------------------------------------
# Kernel body (per core; SPMD -- same program on all 8 cores)
# --------------------------------------------------------------------------
def _emit(ctx: ExitStack, tc: tile.TileContext, tgt: bass.AP, prm: bass.AP,
          loss: bass.AP):
    nc = tc.nc

    iot = ctx.enter_context(tc.tile_pool(name="iot", bufs=4))
    iop = ctx.enter_context(tc.tile_pool(name="iop", bufs=3))
    tp = ctx.enter_context(tc.tile_pool(name="tp", bufs=3))
    tp2 = ctx.enter_context(tc.tile_pool(name="tp2", bufs=2))
    single = ctx.enter_context(tc.tile_pool(name="single", bufs=1))
    psum_pool = ctx.enter_context(
        tc.tile_pool(name="psum", bufs=1, space="PSUM")
    )

    ones = single.tile([128, 1], F32)
    nc.vector.memset(ones[:], 1.0)
    acc = psum_pool.tile([1, PC], F32)

    shb = [128, K, PC]
    for blk in range(NB):
        r0 = blk * RB
        tgv = tgt[r0:r0 + RB, :].rearrange("(k p) w -> p k w", k=K, p=128)
        prv = prm[r0:r0 + RB, :].rearrange("(k p) w -> p k w", k=K, p=128)

        tg = iot.tile([128, K, TGT_W], F32, tag="tg")
        nc.sync.dma_start(tg[:], tgv)
        pr = iop.tile([128, K, PRM_W], F32, tag="pr")
        nc.sync.dma_start(pr[:], prv)

        tg4 = tg[:].rearrange("p k (n c) -> p k n c", c=2)
        pr4 = pr[:].rearrange("p k (n c) -> p k n c", c=5)
        t0v, t1v = tg4[:, :, :, 0], tg4[:, :, :, 1]
        p0v, p1v = pr4[:, :, :, 0], pr4[:, :, :, 1]
        p2v, p3v, p4v = pr4[:, :, :, 2], pr4[:, :, :, 3], pr4[:, :, :, 4]

        isx = tp.tile(shb, BF16, tag="isx")
        isyh = tp.tile(shb, BF16, tag="isyh")
        t4 = tp.tile(shb, F32, tag="t4")
        st = tp.tile(shb, BF16, tag="st")
        t4m1s = tp.tile(shb, BF16, tag="t4m1s")
        t4p1f = tp.tile(shb, F32, tag="t4p1f")
        lvc = tp.tile(shb, BF16, tag="lvc")
        B = tp.tile(shb, BF16, tag="B")      # nyt -> bh
        S = tp.tile(shb, F32, tag="S")       # s1 -> s1b
        A = tp.tile(shb, BF16, tag="A")      # nxt -> a
        G = tp2.tile(shb, BF16, tag="G")     # av -> gv -> gvs
        qn = tp2.tile(shb, BF16, tag="qn")
        W = tp2.tile(shb, BF16, tag="W")
        VN = tp2.tile(shb, F32, tag="VN")    # V -> nll

        # --- ScalarE: Exp-only (single table set) + affines ---
        nc.scalar.activation(t4[:], p4v, AF.Exp, scale=-2.0)
        nc.scalar.activation(t4p1f[:], t4[:], AF.Identity, scale=1.0,
                             bias=1.0)
        nc.scalar.activation(t4m1s[:], t4[:], AF.Identity, scale=SQRT2,
                             bias=-SQRT2)
        nc.scalar.activation(isx[:], p2v, AF.Exp, scale=-1.0)
        nc.scalar.activation(isyh[:], p3v, AF.Exp, scale=-1.0, bias=B_ISYH)
        nc.scalar.activation(st[:], p4v, AF.Exp, scale=1.0, bias=B_ST)
        # lvc = ln(1+t4) - CADD via the exponent-bits log approximation:
        # int32 bits of t4p1f, converted + affine-mapped in one ACTIVATE.
        nc.scalar.activation(lvc[:], t4p1f[:].bitcast(mybir.dt.int32),
                             AF.Identity, scale=LNK, bias=-SIGMA_F * LNK)

        # --- GpSimd: the fp32 strided side-chain ---
        nc.gpsimd.tensor_sub(B[:], t1v, p1v)                  # nyt
        nc.gpsimd.tensor_add(S[:], p2v, p3v)                  # s1
        nc.gpsimd.tensor_sub(S[:], S[:], p4v)                 # s1b

        # --- VectorE ---
        nc.vector.tensor_sub(A[:], t0v, p0v)                  # nxt
        nc.vector.tensor_mul(A[:], A[:], isx[:])              # a
        nc.vector.tensor_mul(B[:], B[:], isyh[:])             # bh
        nc.vector.tensor_mul(G[:], A[:], t4p1f[:])            # av
        nc.vector.tensor_mul(qn[:], B[:], t4m1s[:])
        nc.vector.tensor_add(G[:], G[:], qn[:])               # gv
        nc.vector.tensor_mul(G[:], G[:], st[:])               # gvs
        Wf = W[:].rearrange("p k n -> p (k n)")
        nc.vector._custom_dve(SQ2, out=Wf,
                              in0=G[:].rearrange("p k n -> p (k n)"),
                              in1=B[:].rearrange("p k n -> p (k n)"))
        nc.vector.tensor_sub(VN[:], W[:], lvc[:])             # V
        Vf = VN[:].rearrange("p k n -> p (k n)")
        Sf = S[:].rearrange("p k n -> p (k n)")
        nc.vector._custom_dve(ADDMIN, out=Vf, in0=Vf, in1=Sf, s0=0.0,
                              s1=CLAMP)

        # --- TensorE: frame sum ---
        for k in range(K):
            nc.tensor.matmul(
                acc[:, :], ones[:, :], VN[:, k, :],
                start=(blk == 0 and k == 0),
                stop=(blk == NB - 1 and k == K - 1),
            )

    out_sb = single.tile([1, PC], F32)
    nc.vector.tensor_copy(out_sb[:], acc[:, :])
    nc.sync.dma_start(loss, out_sb[:])


_CACHED_NC = None


def _build_program() -> bass.Bass:
    global _CACHED_NC
    if _CACHED_NC is not None:
        return _CACHED_NC
    nc = bacc.Bacc("TRN2", target_bir_lowering=False, debug=False,
                   enable_asserts=False)
    for v in (B_ISYH, B_ST, -SQRT2, -SIGMA_F * LNK):
        t = nc.alloc_sbuf_tensor(f"const-f32-{v}", [128, 1], F32)
        nc.gpsimd.memset(t.ap(), v)
        nc.const_aps.aps[(F32, v)] = t.ap()
    nc.all_engine_barrier()
    tgt = nc.dram_tensor("tgt", [T, TGT_W], F32, kind="ExternalInput").ap()
    prm = nc.dram_tensor("prm", [T, PRM_W], F32, kind="ExternalInput").ap()
    loss = nc.dram_tensor("loss", [1, PC], F32, kind="ExternalOutput").ap()
    with tile.TileContext(nc) as tc:
        with ExitStack() as ctx:
            _emit(ctx, tc, tgt, prm, loss)
    nc.compile()
    _CACHED_NC = nc
    return nc


def make_in_maps(targets: np.ndarray, params: np.ndarray):
    targets = np.asarray(targets, dtype=np.float32)
    params = np.asarray(params, dtype=np.float32)
    in_maps = []
    for i in range(N_CORES):
        sl = slice(i * PC, (i + 1) * PC)
        in_maps.append({
            "tgt": np.ascontiguousarray(targets[:, sl, :]).reshape(T, TGT_W),
            "prm": np.ascontiguousarray(params[:, sl, :]).reshape(T, PRM_W),
        })
    return in_maps


def run_spmd(targets: np.ndarray, params: np.ndarray, trace: bool = False):
    nc = _build_program()
    in_maps = make_in_maps(targets, params)
    res = bass_utils.run_bass_kernel_spmd(
        nc, in_maps, core_ids=list(range(N_CORES)), trace=trace,
    )
    loss = np.concatenate(
        [res.results[i]["loss"].reshape(PC) for i in range(N_CORES)]
    ).astype(np.float32)
    return loss, res


def kernel(targets: np.ndarray, params: np.ndarray,
           peopleIDs: np.ndarray | None = None) -> np.ndarray:
    loss, _ = run_spmd(targets, params, trace=False)
    return loss
